# revision 33
# baseline (speedup 1.0000x reference)
"""Trainium2 Bass kernel for per-head 3-layer MLP + softmax (nn_Clip).

Reference computation (per head h of 16, batch B=32768):
    h1 = relu(emb @ W1[h] + b1[h])          [B, 128]
    h2 = relu(h1 @ W2[h] + b2[h])           [B, 64]
    out[h] = softmax(h2 @ W3[h] + b3[h])    [B, 10]

Strategy: data-parallel over batch across 8 NeuronCores (4096 rows each),
per-head MLP weights replicated. Layer-1/2 matmuls run in fp8e4 with
DoubleRowSwInterleave (2 contraction rows/cell) and fp32 PSUM
accumulation; layer 3 + softmax run feature-major in bf16.

Layout choices (per core):
  - emb is shipped pre-transposed+fp8 as embT [768, 4096] so layer-1 rhs
    tiles [e_chunk=128, b=512] load contiguously (no on-chip transpose).
  - Layer 1 (feature-major): psum[d1=128, b=512] += W1[h] chunk-pair via
    DoubleRow matmul (lhsT [128,2,128] interleaved, rhs [128,2,512]), 3 MMs
    per head. Dominant cost: ~244ns/MM on HW; 384 MMs ~= 94us of the body.
  - Layer 2: heads paired block-diagonally; head 2j -> psum partitions
    0:64, head 2j+1 -> 64:128; one DoubleRow MM + one relu per pair.
  - Layer 3 stays feature-major (out^T = W3blk^T @ h2): per pair one
    full-width bf16 matmul with a zero-padded stationary block [128,128]
    whose live columns sit at 32*(j%4); the 4 pairs of a bank accumulate
    into one [128,512] psum chain. This replaces the old batch-major form
    whose per-MM 128-column h2 LDWEIGHTS serialized against short
    160-wide MMs. (A tile_position col-tiled variant with [20,512]
    outputs measured far SLOWER on HW despite the concurrency theory.)
  - Softmax feature-major: the head axis lives on psum partitions, so
    b3 folds into the exp as a per-partition activation bias (no bias
    matmuls). One [128,128] selector matmul per bank (M[p',p]=1 iff
    same head) produces the per-head exp sums already broadcast to all
    (h,c) partitions, so the PE appears exactly once in the tail;
    reciprocal_approx_fast on DVE (the exact DVE reciprocal is ~5x
    slower and alone cost ~26us/body); final scale on GpSimd; store is
    transposed ([row, batch]) and the host inverts the row map.
  - Each tile's softmax tail is emitted interleaved into the NEXT tile's
    pair loop so the PE FIFO never waits on ScalarE/VectorE latency.
  - Pools live at program scope (not per body): consecutive bodies in the
    timing loop pipeline through the same buffer rings, so body i+1's
    weight/emb DMAs and first matmuls overlap body i's softmax tail.
"""

import numpy as np
import ml_dtypes
from contextlib import ExitStack

from concourse import bacc, bass, mybir, tile
from concourse.bass_utils import run_bass_kernel_spmd

N_CORES = 8
B = 32768
H = 16
E = 768
D1 = 128
D2 = 64
C = 10
B_LOC = B // N_CORES      # 4096 rows per core
B_TILE = 512              # batch tile (matmul free dim)
N_BT = B_LOC // B_TILE    # 8 tiles per core
KC = E // 128             # 6 contraction chunks for layer 1
NPAIR = H // 2            # 8 head pairs
OUTC = H * C              # 160 logical output columns per row
OROWS = 256               # stored rows: 2 banks x 128 partitions
BODIES_PER_ITER = 2       # bodies per For_i iteration in the timing build

BF16 = mybir.dt.bfloat16
OUT_DT = mybir.dt.bfloat16
F8 = mybir.dt.float8e4
F32 = mybir.dt.float32
AF = mybir.ActivationFunctionType
ALU = mybir.AluOpType

_bf = ml_dtypes.bfloat16
_f8 = ml_dtypes.float8_e4m3

# timing-bisect switches (leave True for correct output)
TAIL_SUMS = True    # sums matmuls + reciprocals
TAIL_BCAST = True   # bcast matmuls + muls
TAIL_STORE = True   # final transposed store


def _io(nc):
    embT = nc.dram_tensor("embT", [E, B_LOC], F8, kind="ExternalInput").ap()
    w1p = nc.dram_tensor("w1p", [128, H * KC * 128], F8,
                         kind="ExternalInput").ap()
    w2p = nc.dram_tensor("w2p", [128, NPAIR * 256], F8,
                         kind="ExternalInput").ap()
    w3p = nc.dram_tensor("w3p", [128, NPAIR * 128], BF16,
                         kind="ExternalInput").ap()
    b1p = nc.dram_tensor("b1p", [128, H], F32, kind="ExternalInput").ap()
    b2p = nc.dram_tensor("b2p", [128, NPAIR], F32, kind="ExternalInput").ap()
    b3p = nc.dram_tensor("b3p", [128, 2], F32, kind="ExternalInput").ap()
    selp = nc.dram_tensor("selp", [128, 128], BF16, kind="ExternalInput").ap()
    out = nc.dram_tensor("out", [OROWS, B_LOC], OUT_DT,
                         kind="ExternalOutput").ap()
    return embT, w1p, w2p, w3p, b1p, b2p, b3p, selp, out


def _make_pools(ctx, tc):
    return dict(
        prog=ctx.enter_context(tc.tile_pool(name="prog", bufs=1)),
        const=ctx.enter_context(tc.tile_pool(name="const", bufs=2)),
        embp=ctx.enter_context(tc.tile_pool(name="embp", bufs=4)),
        h1pool=ctx.enter_context(tc.tile_pool(name="h1pool", bufs=8)),
        h2pool=ctx.enter_context(tc.tile_pool(name="h2pool", bufs=16)),
        smp=ctx.enter_context(tc.tile_pool(name="smp", bufs=2)),
        ps1=ctx.enter_context(tc.tile_pool(name="ps1", bufs=4, space="PSUM")),
        ps2=ctx.enter_context(tc.tile_pool(name="ps2", bufs=2, space="PSUM")),
        ps3=ctx.enter_context(tc.tile_pool(name="ps3", bufs=1, space="PSUM")),
    )


def _prologue(tc, pools):
    """Program-lifetime tiles, psum-zeroing (the feature-major exp reads
    12 unused partitions per 32-block; they must be finite), and HAM
    warmup (PE busy ~3.4us before the first real matmul so the clock
    gate reaches 8/8)."""
    nc = tc.nc
    ones_sb = pools["prog"].tile([1, 128], BF16)
    nc.vector.memset(ones_sb[:], 1.0)
    p3z = pools["ps3"].tile([128, 1024], F32, tag="p3")
    nc.vector.memset(p3z[:], 0.0)
    p_warm = pools["ps1"].tile([128, B_TILE], F32, tag="p1")
    for _ in range(8):
        nc.tensor.matmul(p_warm[:, :64], ones_sb[:1, :], ones_sb[:1, :64],
                         start=True, stop=True)
    return ones_sb


def _body(tc, pools, ones_sb, embT, w1p, w2p, w3p, b1p, b2p, b3p, selp,
          out, pending):
    nc = tc.nc
    const = pools["const"]
    embp = pools["embp"]
    h1pool = pools["h1pool"]
    h2pool = pools["h2pool"]
    smp = pools["smp"]
    ps1 = pools["ps1"]
    ps2 = pools["ps2"]
    ps3 = pools["ps3"]

    embT3 = embT.rearrange("(k e) b -> e k b", e=128)
    # First emb tile loads before the weights on the SP queue so layer 1
    # can start as early as possible; weights follow on the same queue.
    es0 = embp.tile([128, KC, B_TILE], F8, tag="emb")
    nc.sync.dma_start(es0[:], embT3[:, :, 0:B_TILE])
    b1_sb = const.tile([128, H], F32)
    nc.sync.dma_start(b1_sb[:], b1p[:])
    b2_sb = const.tile([128, NPAIR], F32)
    nc.sync.dma_start(b2_sb[:], b2p[:])
    b3_sb = const.tile([128, 2], F32)
    nc.sync.dma_start(b3_sb[:], b3p[:])
    sel_sb = const.tile([128, 128], BF16)
    nc.sync.dma_start(sel_sb[:], selp[:])
    w1_sb = const.tile([128, H * KC, 128], F8)
    w1p3 = w1p[:].rearrange("p (t m) -> p t m", m=128)
    for j in range(NPAIR):
        t0 = 2 * j * KC
        t1 = 2 * (j + 1) * KC
        nc.sync.dma_start(w1_sb[:, t0:t1, :], w1p3[:, t0:t1, :])
    w2_sb = const.tile([128, NPAIR, 256], F8)
    nc.sync.dma_start(w2_sb[:], w2p[:].rearrange("p (j t) -> p j t", t=256))
    w3f_sb = const.tile([128, NPAIR * 128], BF16)
    nc.sync.dma_start(w3f_sb[:], w3p[:])

    outv = out.rearrange("(t p) b -> p t b", p=128)

    def tail_sums(pend):
        # One selector matmul per bank computes the per-head exp sums
        # already broadcast to every (h,c) partition (M[p',p]=1 iff same
        # head); reciprocal lands in SBUF so the psum banks free up after
        # the recip, not after the final scale.
        pbc = ps3.tile([128, 1024], F32, tag="p3")
        pbc3 = pbc[:].rearrange("p (t b) -> p t b", t=2)
        exf = pend["exf"]
        psel = pend["sel_sb"]
        nc.tensor.matmul(pbc3[:, 0, :], psel[:], exf[:, 0, :],
                         start=True, stop=True)
        nc.tensor.matmul(pbc3[:, 1, :], psel[:], exf[:, 1, :],
                         start=True, stop=True)
        # approx reciprocal (~18 bits) is ~5x faster on DVE than the exact
        # one and far above the bf16 output precision anyway; inputs are
        # sums of exps, safely within its defined range.
        rbc = smp.tile([128, 2, B_TILE], F32, tag="rbc")
        nc.vector.reciprocal_approx_fast(rbc[:, 0, :], pbc3[:, 0, :])
        nc.vector.reciprocal_approx_fast(rbc[:, 1, :], pbc3[:, 1, :])
        pend["rbc"] = rbc

    def tail_bcast(pend):
        # final scale, all in SBUF; GpSimd is nearly idle so the muls run
        # there, keeping the DVE queue short for the relus/recips that
        # gate PE psum-bank reuse
        outt = smp.tile([128, 2, B_TILE], OUT_DT, tag="outt")
        nc.gpsimd.tensor_mul(outt[:, 0, :], pend["exf"][:, 0, :],
                             pend["rbc"][:, 0, :])
        nc.gpsimd.tensor_mul(outt[:, 1, :], pend["exf"][:, 1, :],
                             pend["rbc"][:, 1, :])
        pend["outt"] = outt

    def tail_store(pend):
        # store transposed [row, batch]
        outt = pend["outt"]
        bt0 = pend["bt"]
        bsl0 = slice(bt0 * B_TILE, (bt0 + 1) * B_TILE)
        if TAIL_STORE:
            nc.gpsimd.dma_start(outv[:, :, bsl0], outt[:])

    for bt in range(N_BT):
        bsl = slice(bt * B_TILE, (bt + 1) * B_TILE)
        if bt == 0:
            es = es0
        else:
            es = embp.tile([128, KC, B_TILE], F8, tag="emb")
            nc.sync.dma_start(es[:], embT3[:, :, bsl])

        p3f3 = None
        deferred = []
        nrelu = 0
        for j in range(NPAIR):
            if j == 2 and pending is not None and TAIL_BCAST:
                tail_bcast(pending)
            if j == 3:
                if pending is not None:
                    if TAIL_BCAST:
                        tail_store(pending)
                    pending = None

            h1pair = h1pool.tile([128, 2, B_TILE], F8, tag="h1")
            for hi, h in enumerate((2 * j, 2 * j + 1)):
                p1 = ps1.tile([128, B_TILE], F32, tag="p1")
                for k in range(0, KC, 2):
                    nc.tensor.matmul(
                        p1[:],
                        w1_sb[:, h * KC + k:h * KC + k + 2, :],
                        es[:, k:k + 2, :],
                        start=(k == 0),
                        stop=(k == KC - 2),
                        perf_mode=mybir.MatmulPerfMode.DoubleRowSwInterleave,
                    )
                if nrelu % 2 == 0:
                    nc.scalar.activation(h1pair[:, hi, :], p1[:], AF.Relu,
                                         bias=b1_sb[:, h:h + 1])
                else:
                    nc.vector.tensor_scalar(h1pair[:, hi, :], p1[:],
                                            b1_sb[:, h:h + 1],
                                            0.0, ALU.add, ALU.max)
                nrelu += 1

            p2 = ps2.tile([128, B_TILE], F32, tag="p2")
            nc.tensor.matmul(p2[:], w2_sb[:, j, :].rearrange(
                                 "p (t m) -> p t m", m=128),
                             h1pair[:],
                             start=True, stop=True,
                             perf_mode=mybir.MatmulPerfMode.DoubleRowSwInterleave)
            h2 = h2pool.tile([128, B_TILE], BF16, tag="h2")
            if nrelu % 2 == 0:
                nc.scalar.activation(h2[:], p2[:], AF.Relu,
                                     bias=b2_sb[:, j:j + 1])
            else:
                nc.vector.tensor_scalar(h2[:], p2[:], b2_sb[:, j:j + 1],
                                        0.0, ALU.add, ALU.max)
            nrelu += 1

            if j == 1 and pending is not None and TAIL_SUMS:
                tail_sums(pending)

            if j < 4:
                deferred.append((j, h2))
                continue
            if j == 4:
                p3f = ps3.tile([128, 1024], F32, tag="p3")
                p3f3 = p3f[:].rearrange("p (t b) -> p t b", t=2)
                deferred.append((j, h2))
                for dj, dh2 in deferred:
                    t, jj = dj // 4, dj % 4
                    nc.tensor.matmul(
                        p3f3[:, t, :],
                        w3f_sb[:, 128 * dj:128 * (dj + 1)], dh2[:],
                        start=(jj == 0), stop=(jj == 3))
                continue
            t, jj = j // 4, j % 4
            nc.tensor.matmul(
                p3f3[:, t, :],
                w3f_sb[:, 128 * j:128 * (j + 1)], h2[:],
                start=(jj == 0), stop=(jj == 3))

        # exp with the per-partition layer-3 bias fused in
        exf = smp.tile([128, 2, B_TILE], BF16, tag="ex")
        for t in range(2):
            nc.scalar.activation(exf[:, t, :], p3f3[:, t, :], AF.Exp,
                                 bias=b3_sb[:, t:t + 1])
        pending = {"exf": exf, "bt": bt, "sel_sb": sel_sb}
    return pending


def build_program(reps=1):
    nc = bacc.Bacc("TRN2", target_bir_lowering=False, debug=False,
                   num_devices=N_CORES)
    ios = _io(nc)
    with tile.TileContext(nc) as tc:
        with ExitStack() as ctx:
            pools = _make_pools(ctx, tc)
            ones_sb = _prologue(tc, pools)
            pending = None
            for _ in range(reps):
                pending = _body(tc, pools, ones_sb, *ios, pending)
            _body_flush(tc, pools, ios, pending)
    nc.compile()
    return nc


def build_program_loop(iters):
    """Body wrapped in a hardware For_i loop — used only by test.py's
    timing (the loop-count slope cancels the ~70-100ms axon call
    overhead)."""
    nc = bacc.Bacc("TRN2", target_bir_lowering=False, debug=False,
                   num_devices=N_CORES)
    ios = _io(nc)
    with tile.TileContext(nc) as tc:
        with ExitStack() as ctx:
            pools = _make_pools(ctx, tc)
            ones_sb = _prologue(tc, pools)
            with tc.For_i(0, iters, 1):
                pending = None
                for _ in range(BODIES_PER_ITER):
                    pending = _body(tc, pools, ones_sb, *ios, pending)
                _body_flush(tc, pools, ios, pending)
    nc.compile()
    return nc


def _body_flush(tc, pools, ios, pending):
    """Emit the last tile's softmax tail at the end of a body chain."""
    if pending is None or not (TAIL_SUMS and TAIL_BCAST):
        return
    nc = tc.nc
    ps3, smp = pools["ps3"], pools["smp"]
    sel_sb = pending["sel_sb"]
    outv = ios[-1].rearrange("(t p) b -> p t b", p=128)
    exf = pending["exf"]
    pbc = ps3.tile([128, 1024], F32, tag="p3")
    pbc3 = pbc[:].rearrange("p (t b) -> p t b", t=2)
    nc.tensor.matmul(pbc3[:, 0, :], sel_sb[:], exf[:, 0, :],
                     start=True, stop=True)
    nc.tensor.matmul(pbc3[:, 1, :], sel_sb[:], exf[:, 1, :],
                     start=True, stop=True)
    rbc = smp.tile([128, 2, B_TILE], F32, tag="rbc")
    nc.vector.reciprocal_approx_fast(rbc[:, 0, :], pbc3[:, 0, :])
    nc.vector.reciprocal_approx_fast(rbc[:, 1, :], pbc3[:, 1, :])
    outt = smp.tile([128, 2, B_TILE], OUT_DT, tag="outt")
    nc.gpsimd.tensor_mul(outt[:, 0, :], exf[:, 0, :], rbc[:, 0, :])
    nc.gpsimd.tensor_mul(outt[:, 1, :], exf[:, 1, :], rbc[:, 1, :])
    bt0 = pending["bt"]
    bsl0 = slice(bt0 * B_TILE, (bt0 + 1) * B_TILE)
    if TAIL_STORE:
        nc.gpsimd.dma_start(outv[:, :, bsl0], outt[:])


def prep_inputs(clip_embedding, W1, b1, W2, b2, W3, b3):
    """Host-side prepack: cast/transpose into the layouts the kernel DMAs."""
    emb = np.asarray(clip_embedding, dtype=np.float32)
    W1 = np.asarray(W1, dtype=np.float32)
    b1 = np.asarray(b1, dtype=np.float32)
    W2 = np.asarray(W2, dtype=np.float32)
    b2 = np.asarray(b2, dtype=np.float32)
    W3 = np.asarray(W3, dtype=np.float32)
    b3 = np.asarray(b3, dtype=np.float32)

    embT = np.ascontiguousarray(emb.astype(_f8).T)              # [768, B]
    # SwInterleave layout per chunk pair (A=chunk k, B=chunk k+1), stored
    # column order [A127, B127, A126, B126, ..., A0, B0] (see bass_interp).
    w1c = W1.astype(np.float32).reshape(H, KC, 128, D1)          # [h,k,e,d]
    w1p = np.zeros((128, H * KC * D1), dtype=np.float32)
    for h in range(H):
        for kp in range(KC // 2):
            A = w1c[h, 2 * kp]       # [e,d] weights for even chunk
            Bm = w1c[h, 2 * kp + 1]  # [e,d] weights for odd chunk
            blk = np.empty((128, 2 * D1), dtype=np.float32)
            blk[:, 0::2] = A[:, ::-1]
            blk[:, 1::2] = Bm[:, ::-1]
            c0 = (h * KC + 2 * kp) * D1
            w1p[:, c0:c0 + 2 * D1] = blk
    w1p = np.ascontiguousarray(w1p.astype(_f8))
    # Block-diagonal per-pair [256, 128] -> SwInterleave storage [128, 256]:
    # stored col 2t = sub0 col (127-t), col 2t+1 = sub1 col (127-t), where
    # sub0 = [W2[2j] | 0] over d1 of head 2j, sub1 = [0 | W2[2j+1]].
    w2p = np.zeros((128, NPAIR * 256), dtype=np.float32)
    for j in range(NPAIR):
        sub0 = np.zeros((128, 128), dtype=np.float32)
        sub1 = np.zeros((128, 128), dtype=np.float32)
        sub0[:, 0:64] = W2[2 * j]
        sub1[:, 64:128] = W2[2 * j + 1]
        blk = np.empty((128, 256), dtype=np.float32)
        blk[:, 0::2] = sub0[:, ::-1]
        blk[:, 1::2] = sub1[:, ::-1]
        w2p[:, j * 256:(j + 1) * 256] = blk
    w2p = np.ascontiguousarray(w2p.astype(_f8))
    # Feature-major W3: per pair j a [128, 128] stationary block whose
    # columns 32*(j%4)+q hold the pair's block-diagonal W3 (zeros
    # elsewhere); the 4 pairs of a bank accumulate into one full-width
    # psum matmul chain.
    w3p = np.zeros((128, NPAIR * 128), dtype=_bf)
    for j in range(NPAIR):
        base = 128 * j + 32 * (j % 4)
        w3p[0:64, base:base + C] = W3[2 * j].astype(_bf)
        w3p[64:128, base + C:base + 2 * C] = W3[2 * j + 1].astype(_bf)
    b1p = np.ascontiguousarray(b1.T)                            # [128, 16]
    b2p = np.ascontiguousarray(b2.reshape(NPAIR, 128).T)        # [128, 8]
    # Per-partition layer-3 bias for the two psum banks, plus the
    # head-selector matmul operands for the feature-major softmax.
    b3p = np.zeros((128, 2), dtype=np.float32)
    # selp[p', p] = 1 iff partitions p' and p carry the same head; the
    # sum matmul then yields per-head exp sums broadcast to all (h,c)
    # partitions in one shot.
    selp = np.zeros((128, 128), dtype=_bf)
    for jj in range(4):
        for q in range(20):
            p = 32 * jj + q
            s = 2 * jj + q // 10
            c = q % 10
            for t in range(2):
                b3p[p, t] = b3[8 * t + s, c]
            for q2 in range(20):
                if q2 // 10 == q // 10:
                    selp[32 * jj + q2, p] = 1

    shared = dict(w1p=w1p, w2p=w2p, w3p=w3p, b1p=b1p, b2p=b2p, b3p=b3p,
                  selp=selp)
    in_maps = []
    for c in range(N_CORES):
        m = dict(shared)
        m["embT"] = np.ascontiguousarray(
            embT[:, c * B_LOC:(c + 1) * B_LOC])
        in_maps.append(m)
    return in_maps


# out row r = t*128 + 32*jj + 10*k + c  ->  head 8t + 2jj + k, class c
_ROW_MAP = np.zeros((H, C), dtype=np.int64)
for _h in range(H):
    _t, _rem = _h // 8, _h % 8
    _jj, _k = _rem // 2, _rem % 2
    for _c in range(C):
        _ROW_MAP[_h, _c] = _t * 128 + 32 * _jj + 10 * _k + _c


def run(inputs, trace=False):
    """Build, compile and run the SPMD kernel; returns (output, results)."""
    in_maps = prep_inputs(
        inputs["clip_embedding"], inputs["W1"], inputs["b1"],
        inputs["W2"], inputs["b2"], inputs["W3"], inputs["b3"])
    nc = build_program()
    res = run_bass_kernel_spmd(nc, in_maps, list(range(N_CORES)), trace=trace)
    rows = _ROW_MAP.reshape(-1)
    outs = [np.asarray(r["out"], dtype=np.float32)[rows].T for r in res.results]
    full = np.concatenate(outs, axis=0).reshape(B, H, C)
    return full, res


def kernel(**inputs):
    full, _ = run(inputs)
    return full


# revision 38
# speedup vs baseline: 1.0135x; 1.0135x over previous
"""Trainium2 Bass kernel for per-head 3-layer MLP + softmax (nn_Clip).

Reference computation (per head h of 16, batch B=32768):
    h1 = relu(emb @ W1[h] + b1[h])          [B, 128]
    h2 = relu(h1 @ W2[h] + b2[h])           [B, 64]
    out[h] = softmax(h2 @ W3[h] + b3[h])    [B, 10]

Strategy: data-parallel over batch across 8 NeuronCores (4096 rows each),
per-head MLP weights replicated. Layer-1/2 matmuls run in fp8e4 with
DoubleRowSwInterleave (2 contraction rows/cell) and fp32 PSUM
accumulation; layer 3 + softmax run feature-major in bf16.

Layout choices (per core):
  - emb is shipped pre-transposed+fp8 as embT [768, 4096] so layer-1 rhs
    tiles [e_chunk=128, b=512] load contiguously (no on-chip transpose).
  - Layer 1 (feature-major): psum[d1=128, b=512] += W1[h] chunk-pair via
    DoubleRow matmul (lhsT [128,2,128] interleaved, rhs [128,2,512]), 3 MMs
    per head. Dominant cost: ~244ns/MM on HW; 384 MMs ~= 94us of the body.
  - Layer 2: heads paired block-diagonally; head 2j -> psum partitions
    0:64, head 2j+1 -> 64:128; one DoubleRow MM + one relu per pair.
  - Layer 3 stays feature-major (out^T = W3blk^T @ h2): per pair one
    full-width bf16 matmul with a zero-padded stationary block [128,128]
    whose live columns sit at 32*(j%4); the 4 pairs of a bank accumulate
    into one [128,512] psum chain. This replaces the old batch-major form
    whose per-MM 128-column h2 LDWEIGHTS serialized against short
    160-wide MMs. (A tile_position col-tiled variant with [20,512]
    outputs measured far SLOWER on HW despite the concurrency theory.)
  - Softmax feature-major: the head axis lives on psum partitions, so
    b3 folds into the exp as a per-partition activation bias (no bias
    matmuls). One [128,128] selector matmul per bank (M[p',p]=1 iff
    same head) produces the per-head exp sums already broadcast to all
    (h,c) partitions, so the PE appears exactly once in the tail;
    reciprocal_approx_fast on DVE (the exact DVE reciprocal is ~5x
    slower and alone cost ~26us/body); final scale on DVE; store is
    transposed ([row, batch]) and the host inverts the row map.
  - Each tile's softmax tail is emitted interleaved into the NEXT tile's
    pair loop so the PE FIFO never waits on ScalarE/VectorE latency.
  - Pools live at program scope (not per body): consecutive bodies in the
    timing loop pipeline through the same buffer rings, so body i+1's
    weight/emb DMAs and first matmuls overlap body i's softmax tail.
"""

import numpy as np
import ml_dtypes
from contextlib import ExitStack

from concourse import bacc, bass, mybir, tile
from concourse.bass_utils import run_bass_kernel_spmd

N_CORES = 8
B = 32768
H = 16
E = 768
D1 = 128
D2 = 64
C = 10
B_LOC = B // N_CORES      # 4096 rows per core
B_TILE = 512              # batch tile (matmul free dim)
N_BT = B_LOC // B_TILE    # 8 tiles per core
KC = E // 128             # 6 contraction chunks for layer 1
NPAIR = H // 2            # 8 head pairs
OUTC = H * C              # 160 logical output columns per row
OROWS = 256               # stored rows: 2 banks x 128 partitions
BODIES_PER_ITER = 2       # bodies per For_i iteration in the timing build

BF16 = mybir.dt.bfloat16
OUT_DT = mybir.dt.bfloat16
F8 = mybir.dt.float8e4
F32 = mybir.dt.float32
AF = mybir.ActivationFunctionType
ALU = mybir.AluOpType

_bf = ml_dtypes.bfloat16
_f8 = ml_dtypes.float8_e4m3

# timing-bisect switches (leave True for correct output)
TAIL_SUMS = True    # sums matmuls + reciprocals
TAIL_BCAST = True   # bcast matmuls + muls
TAIL_STORE = True   # final transposed store


def _io(nc):
    embT = nc.dram_tensor("embT", [E, B_LOC], F8, kind="ExternalInput").ap()
    w1p = nc.dram_tensor("w1p", [128, H * KC * 128], F8,
                         kind="ExternalInput").ap()
    w2p = nc.dram_tensor("w2p", [128, NPAIR * 256], F8,
                         kind="ExternalInput").ap()
    w3p = nc.dram_tensor("w3p", [128, NPAIR * 128], BF16,
                         kind="ExternalInput").ap()
    b1p = nc.dram_tensor("b1p", [128, H], F32, kind="ExternalInput").ap()
    b2p = nc.dram_tensor("b2p", [128, NPAIR], F32, kind="ExternalInput").ap()
    b3p = nc.dram_tensor("b3p", [128, 2], F32, kind="ExternalInput").ap()
    selp = nc.dram_tensor("selp", [128, 128], BF16, kind="ExternalInput").ap()
    out = nc.dram_tensor("out", [OROWS, B_LOC], OUT_DT,
                         kind="ExternalOutput").ap()
    return embT, w1p, w2p, w3p, b1p, b2p, b3p, selp, out


def _make_pools(ctx, tc):
    return dict(
        prog=ctx.enter_context(tc.tile_pool(name="prog", bufs=1)),
        const=ctx.enter_context(tc.tile_pool(name="const", bufs=2)),
        embp=ctx.enter_context(tc.tile_pool(name="embp", bufs=4)),
        h1pool=ctx.enter_context(tc.tile_pool(name="h1pool", bufs=8)),
        h2pool=ctx.enter_context(tc.tile_pool(name="h2pool", bufs=16)),
        smp=ctx.enter_context(tc.tile_pool(name="smp", bufs=2)),
        ps1=ctx.enter_context(tc.tile_pool(name="ps1", bufs=4, space="PSUM")),
        ps2=ctx.enter_context(tc.tile_pool(name="ps2", bufs=2, space="PSUM")),
        ps3=ctx.enter_context(tc.tile_pool(name="ps3", bufs=1, space="PSUM")),
    )


def _prologue(tc, pools):
    """Program-lifetime tiles, psum-zeroing (the feature-major exp reads
    12 unused partitions per 32-block; they must be finite), and HAM
    warmup (PE busy ~3.4us before the first real matmul so the clock
    gate reaches 8/8)."""
    nc = tc.nc
    ones_sb = pools["prog"].tile([1, 128], BF16)
    nc.vector.memset(ones_sb[:], 1.0)
    p3z = pools["ps3"].tile([128, 1024], F32, tag="p3")
    nc.vector.memset(p3z[:], 0.0)
    p_warm = pools["ps1"].tile([128, B_TILE], F32, tag="p1")
    for _ in range(8):
        nc.tensor.matmul(p_warm[:, :64], ones_sb[:1, :], ones_sb[:1, :64],
                         start=True, stop=True)
    return ones_sb


def _body(tc, pools, ones_sb, embT, w1p, w2p, w3p, b1p, b2p, b3p, selp,
          out, pending):
    nc = tc.nc
    const = pools["const"]
    embp = pools["embp"]
    h1pool = pools["h1pool"]
    h2pool = pools["h2pool"]
    smp = pools["smp"]
    ps1 = pools["ps1"]
    ps2 = pools["ps2"]
    ps3 = pools["ps3"]

    embT3 = embT.rearrange("(k e) b -> e k b", e=128)
    # First emb tile loads before the weights on the SP queue so layer 1
    # can start as early as possible; weights follow on the same queue.
    es0 = embp.tile([128, KC, B_TILE], F8, tag="emb")
    nc.sync.dma_start(es0[:], embT3[:, :, 0:B_TILE])
    b1_sb = const.tile([128, H], F32)
    nc.sync.dma_start(b1_sb[:], b1p[:])
    b2_sb = const.tile([128, NPAIR], F32)
    nc.sync.dma_start(b2_sb[:], b2p[:])
    b3_sb = const.tile([128, 2], F32)
    nc.sync.dma_start(b3_sb[:], b3p[:])
    sel_sb = const.tile([128, 128], BF16)
    nc.sync.dma_start(sel_sb[:], selp[:])
    w1_sb = const.tile([128, H * KC, 128], F8)
    w1p3 = w1p[:].rearrange("p (t m) -> p t m", m=128)
    for j in range(NPAIR):
        t0 = 2 * j * KC
        t1 = 2 * (j + 1) * KC
        nc.sync.dma_start(w1_sb[:, t0:t1, :], w1p3[:, t0:t1, :])
    w2_sb = const.tile([128, NPAIR, 256], F8)
    nc.sync.dma_start(w2_sb[:], w2p[:].rearrange("p (j t) -> p j t", t=256))
    w3f_sb = const.tile([128, NPAIR * 128], BF16)
    nc.sync.dma_start(w3f_sb[:], w3p[:])

    outv = out.rearrange("(t p) b -> p t b", p=128)

    def tail_sums(pend):
        # One selector matmul per bank computes the per-head exp sums
        # already broadcast to every (h,c) partition (M[p',p]=1 iff same
        # head); reciprocal lands in SBUF so the psum banks free up after
        # the recip, not after the final scale.
        pbc = ps3.tile([128, 1024], F32, tag="p3")
        pbc3 = pbc[:].rearrange("p (t b) -> p t b", t=2)
        exf = pend["exf"]
        psel = pend["sel_sb"]
        nc.tensor.matmul(pbc3[:, 0, :], psel[:], exf[:, 0, :],
                         start=True, stop=True)
        nc.tensor.matmul(pbc3[:, 1, :], psel[:], exf[:, 1, :],
                         start=True, stop=True)
        # approx reciprocal (~18 bits) is ~5x faster on DVE than the exact
        # one and far above the bf16 output precision anyway; inputs are
        # sums of exps, safely within its defined range.
        rbc = smp.tile([128, 2, B_TILE], F32, tag="rbc")
        nc.vector.reciprocal_approx_fast(rbc[:, 0, :], pbc3[:, 0, :])
        nc.vector.reciprocal_approx_fast(rbc[:, 1, :], pbc3[:, 1, :])
        pend["rbc"] = rbc

    def tail_bcast(pend):
        # final scale, all in SBUF (measured faster on DVE than GpSimd)
        outt = smp.tile([128, 2, B_TILE], OUT_DT, tag="outt")
        nc.vector.tensor_mul(outt[:, 0, :], pend["exf"][:, 0, :],
                             pend["rbc"][:, 0, :])
        nc.vector.tensor_mul(outt[:, 1, :], pend["exf"][:, 1, :],
                             pend["rbc"][:, 1, :])
        pend["outt"] = outt

    def tail_store(pend):
        # store transposed [row, batch]
        outt = pend["outt"]
        bt0 = pend["bt"]
        bsl0 = slice(bt0 * B_TILE, (bt0 + 1) * B_TILE)
        if TAIL_STORE:
            nc.gpsimd.dma_start(outv[:, :, bsl0], outt[:])

    for bt in range(N_BT):
        bsl = slice(bt * B_TILE, (bt + 1) * B_TILE)
        if bt == 0:
            es = es0
        else:
            es = embp.tile([128, KC, B_TILE], F8, tag="emb")
            nc.sync.dma_start(es[:], embT3[:, :, bsl])

        p3f3 = None
        deferred = []
        nrelu = 0
        for j in range(NPAIR):
            if j == 2 and pending is not None and TAIL_BCAST:
                tail_bcast(pending)
            if j == 3:
                if pending is not None:
                    if TAIL_BCAST:
                        tail_store(pending)
                    pending = None

            h1pair = h1pool.tile([128, 2, B_TILE], F8, tag="h1")
            for hi, h in enumerate((2 * j, 2 * j + 1)):
                p1 = ps1.tile([128, B_TILE], F32, tag="p1")
                for k in range(0, KC, 2):
                    nc.tensor.matmul(
                        p1[:],
                        w1_sb[:, h * KC + k:h * KC + k + 2, :],
                        es[:, k:k + 2, :],
                        start=(k == 0),
                        stop=(k == KC - 2),
                        perf_mode=mybir.MatmulPerfMode.DoubleRowSwInterleave,
                    )
                if nrelu % 2 == 0:
                    nc.scalar.activation(h1pair[:, hi, :], p1[:], AF.Relu,
                                         bias=b1_sb[:, h:h + 1])
                else:
                    nc.vector.tensor_scalar(h1pair[:, hi, :], p1[:],
                                            b1_sb[:, h:h + 1],
                                            0.0, ALU.add, ALU.max)
                nrelu += 1

            p2 = ps2.tile([128, B_TILE], F32, tag="p2")
            nc.tensor.matmul(p2[:], w2_sb[:, j, :].rearrange(
                                 "p (t m) -> p t m", m=128),
                             h1pair[:],
                             start=True, stop=True,
                             perf_mode=mybir.MatmulPerfMode.DoubleRowSwInterleave)
            h2 = h2pool.tile([128, B_TILE], BF16, tag="h2")
            # L2 relus gate the L3 matmuls; ScalarE is faster per relu and
            # lighter loaded (DVE also runs the recips/muls), so only 2 of
            # 8 go to DVE: 14 Act / 10 DVE ops per tile, ~balanced.
            if j % 4 != 3:
                nc.scalar.activation(h2[:], p2[:], AF.Relu,
                                     bias=b2_sb[:, j:j + 1])
            else:
                nc.vector.tensor_scalar(h2[:], p2[:], b2_sb[:, j:j + 1],
                                        0.0, ALU.add, ALU.max)
            nrelu += 1

            if j == 1 and pending is not None and TAIL_SUMS:
                tail_sums(pending)

            if j < 4:
                deferred.append((j, h2))
                continue
            if j == 4:
                p3f = ps3.tile([128, 1024], F32, tag="p3")
                p3f3 = p3f[:].rearrange("p (t b) -> p t b", t=2)
                deferred.append((j, h2))
                for dj, dh2 in deferred:
                    t, jj = dj // 4, dj % 4
                    nc.tensor.matmul(
                        p3f3[:, t, :],
                        w3f_sb[:, 128 * dj:128 * (dj + 1)], dh2[:],
                        start=(jj == 0), stop=(jj == 3))
                # bank A logits are complete: exp it now (with the fused
                # per-partition layer-3 bias) instead of in the tile-end
                # burst, spreading ScalarE load
                exf = smp.tile([128, 2, B_TILE], BF16, tag="ex")
                nc.scalar.activation(exf[:, 0, :], p3f3[:, 0, :], AF.Exp,
                                     bias=b3_sb[:, 0:1])
                continue
            t, jj = j // 4, j % 4
            nc.tensor.matmul(
                p3f3[:, t, :],
                w3f_sb[:, 128 * j:128 * (j + 1)], h2[:],
                start=(jj == 0), stop=(jj == 3))

        nc.scalar.activation(exf[:, 1, :], p3f3[:, 1, :], AF.Exp,
                             bias=b3_sb[:, 1:2])
        pending = {"exf": exf, "bt": bt, "sel_sb": sel_sb}
    return pending


def build_program(reps=1):
    nc = bacc.Bacc("TRN2", target_bir_lowering=False, debug=False,
                   num_devices=N_CORES)
    ios = _io(nc)
    with tile.TileContext(nc) as tc:
        with ExitStack() as ctx:
            pools = _make_pools(ctx, tc)
            ones_sb = _prologue(tc, pools)
            pending = None
            for _ in range(reps):
                pending = _body(tc, pools, ones_sb, *ios, pending)
            _body_flush(tc, pools, ios, pending)
    nc.compile()
    return nc


def build_program_loop(iters):
    """Body wrapped in a hardware For_i loop — used only by test.py's
    timing (the loop-count slope cancels the ~70-100ms axon call
    overhead)."""
    nc = bacc.Bacc("TRN2", target_bir_lowering=False, debug=False,
                   num_devices=N_CORES)
    ios = _io(nc)
    with tile.TileContext(nc) as tc:
        with ExitStack() as ctx:
            pools = _make_pools(ctx, tc)
            ones_sb = _prologue(tc, pools)
            with tc.For_i(0, iters, 1):
                pending = None
                for _ in range(BODIES_PER_ITER):
                    pending = _body(tc, pools, ones_sb, *ios, pending)
                _body_flush(tc, pools, ios, pending)
    nc.compile()
    return nc


def _body_flush(tc, pools, ios, pending):
    """Emit the last tile's softmax tail at the end of a body chain."""
    if pending is None or not (TAIL_SUMS and TAIL_BCAST):
        return
    nc = tc.nc
    ps3, smp = pools["ps3"], pools["smp"]
    sel_sb = pending["sel_sb"]
    outv = ios[-1].rearrange("(t p) b -> p t b", p=128)
    exf = pending["exf"]
    pbc = ps3.tile([128, 1024], F32, tag="p3")
    pbc3 = pbc[:].rearrange("p (t b) -> p t b", t=2)
    nc.tensor.matmul(pbc3[:, 0, :], sel_sb[:], exf[:, 0, :],
                     start=True, stop=True)
    nc.tensor.matmul(pbc3[:, 1, :], sel_sb[:], exf[:, 1, :],
                     start=True, stop=True)
    rbc = smp.tile([128, 2, B_TILE], F32, tag="rbc")
    nc.vector.reciprocal_approx_fast(rbc[:, 0, :], pbc3[:, 0, :])
    nc.vector.reciprocal_approx_fast(rbc[:, 1, :], pbc3[:, 1, :])
    outt = smp.tile([128, 2, B_TILE], OUT_DT, tag="outt")
    nc.vector.tensor_mul(outt[:, 0, :], exf[:, 0, :], rbc[:, 0, :])
    nc.vector.tensor_mul(outt[:, 1, :], exf[:, 1, :], rbc[:, 1, :])
    bt0 = pending["bt"]
    bsl0 = slice(bt0 * B_TILE, (bt0 + 1) * B_TILE)
    if TAIL_STORE:
        nc.gpsimd.dma_start(outv[:, :, bsl0], outt[:])


def prep_inputs(clip_embedding, W1, b1, W2, b2, W3, b3):
    """Host-side prepack: cast/transpose into the layouts the kernel DMAs."""
    emb = np.asarray(clip_embedding, dtype=np.float32)
    W1 = np.asarray(W1, dtype=np.float32)
    b1 = np.asarray(b1, dtype=np.float32)
    W2 = np.asarray(W2, dtype=np.float32)
    b2 = np.asarray(b2, dtype=np.float32)
    W3 = np.asarray(W3, dtype=np.float32)
    b3 = np.asarray(b3, dtype=np.float32)

    embT = np.ascontiguousarray(emb.astype(_f8).T)              # [768, B]
    # SwInterleave layout per chunk pair (A=chunk k, B=chunk k+1), stored
    # column order [A127, B127, A126, B126, ..., A0, B0] (see bass_interp).
    w1c = W1.astype(np.float32).reshape(H, KC, 128, D1)          # [h,k,e,d]
    w1p = np.zeros((128, H * KC * D1), dtype=np.float32)
    for h in range(H):
        for kp in range(KC // 2):
            A = w1c[h, 2 * kp]       # [e,d] weights for even chunk
            Bm = w1c[h, 2 * kp + 1]  # [e,d] weights for odd chunk
            blk = np.empty((128, 2 * D1), dtype=np.float32)
            blk[:, 0::2] = A[:, ::-1]
            blk[:, 1::2] = Bm[:, ::-1]
            c0 = (h * KC + 2 * kp) * D1
            w1p[:, c0:c0 + 2 * D1] = blk
    w1p = np.ascontiguousarray(w1p.astype(_f8))
    # Block-diagonal per-pair [256, 128] -> SwInterleave storage [128, 256]:
    # stored col 2t = sub0 col (127-t), col 2t+1 = sub1 col (127-t), where
    # sub0 = [W2[2j] | 0] over d1 of head 2j, sub1 = [0 | W2[2j+1]].
    w2p = np.zeros((128, NPAIR * 256), dtype=np.float32)
    for j in range(NPAIR):
        sub0 = np.zeros((128, 128), dtype=np.float32)
        sub1 = np.zeros((128, 128), dtype=np.float32)
        sub0[:, 0:64] = W2[2 * j]
        sub1[:, 64:128] = W2[2 * j + 1]
        blk = np.empty((128, 256), dtype=np.float32)
        blk[:, 0::2] = sub0[:, ::-1]
        blk[:, 1::2] = sub1[:, ::-1]
        w2p[:, j * 256:(j + 1) * 256] = blk
    w2p = np.ascontiguousarray(w2p.astype(_f8))
    # Feature-major W3: per pair j a [128, 128] stationary block whose
    # columns 32*(j%4)+q hold the pair's block-diagonal W3 (zeros
    # elsewhere); the 4 pairs of a bank accumulate into one full-width
    # psum matmul chain.
    w3p = np.zeros((128, NPAIR * 128), dtype=_bf)
    for j in range(NPAIR):
        base = 128 * j + 32 * (j % 4)
        w3p[0:64, base:base + C] = W3[2 * j].astype(_bf)
        w3p[64:128, base + C:base + 2 * C] = W3[2 * j + 1].astype(_bf)
    b1p = np.ascontiguousarray(b1.T)                            # [128, 16]
    b2p = np.ascontiguousarray(b2.reshape(NPAIR, 128).T)        # [128, 8]
    # Per-partition layer-3 bias for the two psum banks, plus the
    # head-selector matmul operands for the feature-major softmax.
    b3p = np.zeros((128, 2), dtype=np.float32)
    # selp[p', p] = 1 iff partitions p' and p carry the same head; the
    # sum matmul then yields per-head exp sums broadcast to all (h,c)
    # partitions in one shot.
    selp = np.zeros((128, 128), dtype=_bf)
    for jj in range(4):
        for q in range(20):
            p = 32 * jj + q
            s = 2 * jj + q // 10
            c = q % 10
            for t in range(2):
                b3p[p, t] = b3[8 * t + s, c]
            for q2 in range(20):
                if q2 // 10 == q // 10:
                    selp[32 * jj + q2, p] = 1

    shared = dict(w1p=w1p, w2p=w2p, w3p=w3p, b1p=b1p, b2p=b2p, b3p=b3p,
                  selp=selp)
    in_maps = []
    for c in range(N_CORES):
        m = dict(shared)
        m["embT"] = np.ascontiguousarray(
            embT[:, c * B_LOC:(c + 1) * B_LOC])
        in_maps.append(m)
    return in_maps


# out row r = t*128 + 32*jj + 10*k + c  ->  head 8t + 2jj + k, class c
_ROW_MAP = np.zeros((H, C), dtype=np.int64)
for _h in range(H):
    _t, _rem = _h // 8, _h % 8
    _jj, _k = _rem // 2, _rem % 2
    for _c in range(C):
        _ROW_MAP[_h, _c] = _t * 128 + 32 * _jj + 10 * _k + _c


def run(inputs, trace=False):
    """Build, compile and run the SPMD kernel; returns (output, results)."""
    in_maps = prep_inputs(
        inputs["clip_embedding"], inputs["W1"], inputs["b1"],
        inputs["W2"], inputs["b2"], inputs["W3"], inputs["b3"])
    nc = build_program()
    res = run_bass_kernel_spmd(nc, in_maps, list(range(N_CORES)), trace=trace)
    rows = _ROW_MAP.reshape(-1)
    outs = [np.asarray(r["out"], dtype=np.float32)[rows].T for r in res.results]
    full = np.concatenate(outs, axis=0).reshape(B, H, C)
    return full, res


def kernel(**inputs):
    full, _ = run(inputs)
    return full
